# revision 1
# baseline (speedup 1.0000x reference)
"""Trainium2 Bass kernel for MinecraftAwareAttention3D.

Full (unsharded) inputs in, full output out. Internally: one attention head
per NeuronCore (tensor parallel over num_heads=8), GroupNorm + QKV replicated
per core, per-head projection partials summed on the host.

Key tricks:
  * The material / vertical boosts are bilinear in small per-position feature
    vectors, so they are folded into the QK^T matmul as 18 extra contraction
    rows (Q' = [q*scale; L], K' = [k; R] -> S = Q'.K' = qk*scale + boost).
  * Air keys are compacted away on the host (softmax weight for them is
    exactly 0), shrinking the key axis from 4096 to ~3072.
  * No max-subtraction in softmax (logits are O(10), exp cannot overflow);
    exp of padded key rows is killed with a -1e9 per-partition ACT bias.
  * A ones-row appended to V^T makes the PV matmul also produce the softmax
    denominator.
  * float32r (full-rate fp32 matmul mode) everywhere on the PE.
"""

import numpy as np

import concourse.bass as bass
import concourse.tile as tile
from concourse import mybir
from concourse.bass_utils import run_bass_kernel_spmd

F32 = mybir.dt.float32
F32R = mybir.dt.float32r
AF = mybir.ActivationFunctionType
ALU = mybir.AluOpType

B, C, D_, H_, W_ = 1, 256, 16, 16, 16
N = D_ * H_ * W_          # 4096 spatial positions
HEADS, HD = 8, 32
GROUPS = 8                # groupnorm groups -> 32 channels per group
GSIZE = C // GROUPS
EPS = 1e-5
NEG = -1e9
NF = HD + 18              # fused contraction depth: 32 qk dims + 18 boost dims
NCORES = 8

TRACE = False             # test.py can flip this for profiling
LAST_RESULT = {}

_CACHE = {}


def _split_waits(nc, max_waits=1):
    """This walrus build only encodes one sync wait per instruction; hoist
    extra waits onto same-engine NOPs inserted just before the instruction."""
    n = 0
    for f in nc.m.functions:
        for bb in f.blocks:
            new_insts = []
            for inst in bb.instructions:
                si = inst.sync_info
                if si is not None and si.on_wait and len(si.on_wait) > max_waits:
                    waits = list(si.on_wait)
                    si.on_wait = waits[-max_waits:]
                    for i in range(0, len(waits) - max_waits, max_waits):
                        n += 1
                        nop = mybir.InstNoOp(name=f"I-wsplit-{n}", ins=[], outs=[])
                        nop.engine = inst.engine
                        nop.sync_info = mybir.SyncInfo(
                            on_wait=waits[i : i + max_waits], on_update=[]
                        )
                        new_insts.append(nop)
                new_insts.append(inst)
            bb.instructions[:] = new_insts
    return n


def _build(nk_pad):
    """Build the per-core Bass module. Static on the padded compacted key
    count; all data arrives as ExternalInputs."""
    nkc = nk_pad // 128                       # 128-key chunks
    ksl = [min(512, nk_pad - s) for s in range(0, nk_pad, 512)]  # kv col chunks

    nc = bass.Bass()

    # ---- I/O ----
    x2 = nc.dram_tensor("x2", [C, N], F32, kind="ExternalInput")
    xc = nc.dram_tensor("xc", [C, nk_pad], F32, kind="ExternalInput")
    lfeat = nc.dram_tensor("lfeat", [18, N], F32R, kind="ExternalInput")
    rfeat = nc.dram_tensor("rfeat", [18, nk_pad], F32R, kind="ExternalInput")
    abias = nc.dram_tensor("abias", [128, nkc], F32, kind="ExternalInput")
    gseli = nc.dram_tensor("gseli", [128, 4], F32R, kind="ExternalInput")
    gselt = nc.dram_tensor("gselt", [4, 128], F32R, kind="ExternalInput")
    ident = nc.dram_tensor("ident", [33, 33], F32, kind="ExternalInput")
    one32 = nc.dram_tensor("one32", [1, HD], F32R, kind="ExternalInput")
    onesk = nc.dram_tensor("onesk", [1, nk_pad], F32, kind="ExternalInput")
    gnw = nc.dram_tensor("gnw", [C, 1], F32, kind="ExternalInput")
    gnb = nc.dram_tensor("gnb", [C, 1], F32, kind="ExternalInput")
    wq = nc.dram_tensor("wq", [C, HD], F32R, kind="ExternalInput")
    wk = nc.dram_tensor("wk", [C, HD], F32R, kind="ExternalInput")
    wv = nc.dram_tensor("wv", [C, HD], F32R, kind="ExternalInput")
    bq = nc.dram_tensor("bq", [HD, 1], F32, kind="ExternalInput")
    bk = nc.dram_tensor("bk", [HD, 1], F32, kind="ExternalInput")
    bv = nc.dram_tensor("bv", [HD, 1], F32, kind="ExternalInput")
    pwt = nc.dram_tensor("pwt", [HD, C], F32R, kind="ExternalInput")
    out = nc.dram_tensor("o", [C, N], F32, kind="ExternalOutput")

    with tile.TileContext(nc) as tc:
        with (
            tc.tile_pool(name="consts", bufs=1) as cp,
            tc.tile_pool(name="live", bufs=1) as lp,
            tc.tile_pool(name="small", bufs=2) as sp,
            tc.tile_pool(name="ps_small", bufs=2, space="PSUM") as pss,
            tc.tile_pool(name="ps_s", bufs=2, space="PSUM") as ps_s,
            tc.tile_pool(name="ps_pv", bufs=1, space="PSUM") as ps_pv,
        ):
            # ---- long-lived activations ----
            h = [lp.tile([128, N], F32R, name=f"h{c}") for c in range(2)]
            hk = [lp.tile([128, nk_pad], F32R, name=f"hk{c}") for c in range(2)]
            qf = lp.tile([NF, N], F32R)           # Q' = [q*scale ; L]
            kf = lp.tile([NF, nk_pad], F32R)      # K' = [k ; R]
            vv = lp.tile([HD + 1, nk_pad], F32)   # [v ; ones]
            vt = lp.tile([128, nkc, HD + 1], F32R)  # per-chunk V'^T

            # Warm the ACT exp table-set before anything else touches ACT:
            # every later ACT op (Identity/Copy adds, Exp) then runs from the
            # already-resident set with no mid-kernel table switch.
            wz = cp.tile([1, 1], F32)
            nc.vector.memset(wz, 0.0)
            wy = cp.tile([1, 1], F32)
            nc.scalar.activation(out=wy, in_=wz, func=AF.Exp, bias=0.0, scale=1.0)

            # ================= Phase 1: GroupNorm =================
            with tc.tile_pool(name="xpool", bufs=1) as xp:
                # x loads go first on the HWDGE queues: they gate the whole
                # GroupNorm -> QKV -> attention chain.
                xt = [xp.tile([128, N], F32, name=f"xt{c}") for c in range(2)]
                for c in range(2):
                    for s in range(4):
                        nc.sync.dma_start(
                            out=xt[c][:, s * 1024 : (s + 1) * 1024],
                            in_=x2[c * 128 : (c + 1) * 128, s * 1024 : (s + 1) * 1024],
                        )

                # ---- constants (SWDGE queues, off the critical path) ----
                gsel_t = cp.tile([128, 4], F32R)
                nc.gpsimd.dma_start(out=gsel_t, in_=gseli[:, :])
                gselt_t = cp.tile([4, 128], F32R)
                nc.gpsimd.dma_start(out=gselt_t, in_=gselt[:, :])
                ident_t = cp.tile([33, 33], F32)
                nc.gpsimd.dma_start(out=ident_t, in_=ident[:, :])
                gnw_t = cp.tile([C // 2, 2], F32)
                nc.gpsimd.dma_start(out=gnw_t[:, 0:1], in_=gnw[0:128, :])
                nc.gpsimd.dma_start(out=gnw_t[:, 1:2], in_=gnw[128:256, :])
                gnb_t = cp.tile([C // 2, 2], F32)
                nc.gpsimd.dma_start(out=gnb_t[:, 0:1], in_=gnb[0:128, :])
                nc.gpsimd.dma_start(out=gnb_t[:, 1:2], in_=gnb[128:256, :])
                wq_t = cp.tile([128, 2, HD], F32R)
                wk_t = cp.tile([128, 2, HD], F32R)
                wv_t = cp.tile([128, 2, HD], F32R)
                for t, d in ((wq_t, wq), (wk_t, wk), (wv_t, wv)):
                    nc.gpsimd.dma_start(out=t[:, 0, :], in_=d[0:128, :])
                    nc.gpsimd.dma_start(out=t[:, 1, :], in_=d[128:256, :])
                bq_t = cp.tile([HD, 1], F32)
                nc.gpsimd.dma_start(out=bq_t, in_=bq[:, :])
                bk_t = cp.tile([HD, 1], F32)
                nc.gpsimd.dma_start(out=bk_t, in_=bk[:, :])
                bv_t = cp.tile([HD, 1], F32)
                nc.gpsimd.dma_start(out=bv_t, in_=bv[:, :])
                pwt_t = cp.tile([HD, C], F32R)
                nc.gpsimd.dma_start(out=pwt_t, in_=pwt[:, :])
                abias_t = cp.tile([128, nkc], F32)
                nc.gpsimd.dma_start(out=abias_t, in_=abias[:, :])
                ones_t = cp.tile([1, HD], F32R)
                nc.gpsimd.dma_start(out=ones_t, in_=one32[:, :])
                nc.gpsimd.dma_start(out=qf[HD:NF, :], in_=lfeat[:, :])
                nc.gpsimd.dma_start(out=kf[HD:NF, :], in_=rfeat[:, :])
                nc.gpsimd.dma_start(out=vv[HD : HD + 1, :], in_=onesk[:, :])

                stats2 = sp.tile([128, 4], F32R)  # [mean_c0, E2_c0, mean_c1, E2_c1]
                for c in range(2):
                    st6 = sp.tile([128, 8, 6], F32, name=f"st6_{c}")
                    for s in range(8):
                        nc.vector.bn_stats(
                            out=st6[:, s, :], in_=xt[c][:, s * 512 : (s + 1) * 512]
                        )
                    mv = sp.tile([128, 2], F32, name=f"mv_{c}")
                    nc.vector.bn_aggr(out=mv, in_=st6)
                    m2 = sp.tile([128, 1], F32, name=f"m2_{c}")
                    nc.vector.tensor_mul(out=m2, in0=mv[:, 0:1], in1=mv[:, 0:1])
                    nc.vector.tensor_copy(
                        out=stats2[:, 2 * c : 2 * c + 1], in_=mv[:, 0:1]
                    )
                    nc.vector.tensor_add(
                        out=stats2[:, 2 * c + 1 : 2 * c + 2], in0=mv[:, 1:2], in1=m2
                    )

                gstat = pss.tile([4, 4], F32, space="PSUM", tag="s")
                nc.tensor.matmul(gstat, lhsT=gsel_t, rhs=stats2, start=True, stop=True)

                ab = []
                for c in range(2):
                    mu4 = sp.tile([4, 1], F32R, name=f"mu4_{c}")
                    nc.vector.tensor_scalar_mul(
                        out=mu4, in0=gstat[:, 2 * c : 2 * c + 1], scalar1=1.0 / GSIZE
                    )
                    ve = sp.tile([4, 1], F32, name=f"ve_{c}")
                    nc.vector.tensor_scalar_mul(
                        out=ve, in0=gstat[:, 2 * c + 1 : 2 * c + 2], scalar1=1.0 / GSIZE
                    )
                    mum = sp.tile([4, 1], F32, name=f"mum_{c}")
                    nc.vector.tensor_mul(out=mum, in0=mu4, in1=mu4)
                    nc.vector.tensor_sub(out=ve, in0=ve, in1=mum)
                    nc.vector.tensor_scalar_add(out=ve, in0=ve, scalar1=EPS)
                    # rsqrt without ACT (avoids the sqrt table-set load):
                    # quake seed y = bits(0x5f3759df - (i >> 1)), then 3
                    # Newton steps y' = y * (1.5 - 0.5*v*y^2)
                    I32 = mybir.dt.int32
                    yi = sp.tile([4, 1], I32, name=f"yi_{c}")
                    nc.vector.tensor_scalar(
                        out=yi, in0=ve.bitcast(I32), scalar1=1, scalar2=None,
                        op0=ALU.logical_shift_right,
                    )
                    nc.vector.tensor_scalar(
                        out=yi, in0=yi, scalar1=-1, scalar2=0x5F3759DF,
                        op0=ALU.mult, op1=ALU.add,
                    )
                    rs = sp.tile([4, 1], F32, name=f"rs_{c}")
                    nc.vector.tensor_copy(out=rs, in_=yi.bitcast(F32))
                    t2 = sp.tile([4, 1], F32, name=f"t2_{c}")
                    for it in range(3):
                        nc.vector.tensor_mul(out=t2, in0=rs, in1=rs)
                        nc.vector.tensor_mul(out=t2, in0=t2, in1=ve)
                        nc.vector.tensor_scalar(
                            out=t2, in0=t2, scalar1=-0.5, scalar2=1.5,
                            op0=ALU.mult, op1=ALU.add,
                        )
                        nc.vector.tensor_mul(out=rs, in0=rs, in1=t2)
                    rs2 = sp.tile([4, 1], F32R, name=f"rs2_{c}")
                    nc.vector.tensor_copy(out=rs2, in_=rs)

                    musig = sp.tile([4, 2], F32R, name=f"musig_{c}")
                    nc.vector.tensor_copy(out=musig[:, 0:1], in_=mu4)
                    nc.vector.tensor_copy(out=musig[:, 1:2], in_=rs2)
                    bc = pss.tile([128, 2], F32, space="PSUM", name=f"bc_{c}", tag="s")
                    nc.tensor.matmul(bc, lhsT=gselt_t, rhs=musig, start=True, stop=True)
                    # a = gn_w * rstd ; b = gn_b - mu * a
                    a_ch = sp.tile([128, 1], F32, name=f"a_ch_{c}")
                    nc.vector.tensor_mul(out=a_ch, in0=gnw_t[:, c : c + 1], in1=bc[:, 1:2])
                    b_ch = sp.tile([128, 1], F32, name=f"b_ch_{c}")
                    nc.vector.tensor_mul(out=b_ch, in0=bc[:, 0:1], in1=a_ch)
                    nc.vector.tensor_sub(out=b_ch, in0=gnb_t[:, c : c + 1], in1=b_ch)
                    ab.append((a_ch, b_ch))

                    # first query half only: unblocks q-matmuls for the
                    # first two q-groups; second half follows off-path
                    nc.vector.tensor_scalar(
                        out=h[c][:, 0 : N // 2], in0=xt[c][:, 0 : N // 2],
                        scalar1=a_ch, scalar2=b_ch, op0=ALU.mult, op1=ALU.add,
                    )
                # compacted keys: stream xc and normalize into hk
                # (slice-major so early key slices finish first)
                for s0 in range(0, nk_pad, 1024):
                    s1 = min(s0 + 1024, nk_pad)
                    for c in range(2):
                        a_ch, b_ch = ab[c]
                        xs_t = xp.tile([128, 1024], F32, name="xcs", tag="xcs", bufs=4)
                        nc.sync.dma_start(
                            out=xs_t[:, 0 : s1 - s0],
                            in_=xc[c * 128 : (c + 1) * 128, s0:s1],
                        )
                        nc.vector.tensor_scalar(
                            out=hk[c][:, s0:s1], in0=xs_t[:, 0 : s1 - s0],
                            scalar1=a_ch, scalar2=b_ch, op0=ALU.mult, op1=ALU.add,
                        )
                for c in range(2):
                    a_ch, b_ch = ab[c]
                    nc.vector.tensor_scalar(
                        out=h[c][:, N // 2 :], in0=xt[c][:, N // 2 :],
                        scalar1=a_ch, scalar2=b_ch, op0=ALU.mult, op1=ALU.add,
                    )

            with (
                tc.tile_pool(name="pbig", bufs=3) as pb,
                tc.tile_pool(name="opool", bufs=3) as op,
            ):
                # ================= Phase 2: QKV + V transpose =================
                ksn = len(ksl)
                for i in range(max(8, ksn)):
                    paths = []
                    if i < ksn:
                        paths.append((wk_t, bk_t, kf, hk, ksl[i], i * 512))
                    if i < 8:
                        paths.append((wq_t, bq_t, qf, h, 512, i * 512))
                    if i < ksn:
                        paths.append((wv_t, bv_t, vv, hk, ksl[i], i * 512))
                    for j, (w_t, b_t, dst, src_t, w_n, s0) in enumerate(paths):
                        sl = slice(s0, s0 + w_n)
                        # borrow the (idle) attention PSUM slots for extra
                        # buffering so matmuls are not slot-gated on copies
                        pool = ps_s if j % 2 == 0 else pss
                        tag = "st" if j % 2 == 0 else "s"
                        ps = pool.tile([HD, 512], F32, space="PSUM", name="qkv_ps", tag=tag)
                        nc.tensor.matmul(
                            ps[:, 0:w_n], lhsT=w_t[:, 0, :], rhs=src_t[0][:, sl],
                            start=True, stop=False,
                        )
                        nc.tensor.matmul(
                            ps[:, 0:w_n], lhsT=w_t[:, 1, :], rhs=src_t[1][:, sl],
                            start=False, stop=True,
                        )
                        if dst is kf:  # k on DVE; q/v on the idle ACT engine
                            nc.vector.tensor_scalar_add(
                                out=dst[0:HD, sl], in0=ps[:, 0:w_n], scalar1=b_t
                            )
                        else:
                            nc.scalar.add(
                                out=dst[0:HD, sl], in_=ps[:, 0:w_n], add=b_t
                            )

                for kc in range(nkc):
                    tps = pss.tile([128, HD + 1], F32, space="PSUM", name="tr_ps", tag="s")
                    nc.tensor.transpose(
                        tps, in_=vv[:, kc * 128 : (kc + 1) * 128], identity=ident_t
                    )
                    nc.vector.tensor_copy(out=vt[:, kc, :], in_=tps)

                # ========== Phase 3: attention (+ fused projection) ==========
                for qg in range(4):
                    q0 = qg * 1024
                    pv = ps_pv.tile([HD + 1, 1024], F32, space="PSUM", name="pv")
                    for kc in range(nkc):
                        st = ps_s.tile([128, 1024], F32, space="PSUM", name="st")
                        lhs = kf[:, kc * 128 : (kc + 1) * 128]
                        nc.tensor.matmul(
                            st[:, 0:512], lhsT=lhs, rhs=qf[:, q0 : q0 + 512],
                            start=True, stop=True,
                        )
                        nc.tensor.matmul(
                            st[:, 512:1024], lhsT=lhs, rhs=qf[:, q0 + 512 : q0 + 1024],
                            start=True, stop=True,
                        )
                        pt = pb.tile([128, 1024], F32R, name="pt")
                        nc.scalar.activation(
                            out=pt, in_=st, func=AF.Exp,
                            bias=abias_t[:, kc : kc + 1], scale=1.0,
                        )
                        vlhs = vt[:, kc, :]
                        nc.tensor.matmul(
                            pv[:, 0:512], lhsT=vlhs, rhs=pt[:, 0:512],
                            start=(kc == 0), stop=(kc == nkc - 1), skip_group_check=True,
                        )
                        nc.tensor.matmul(
                            pv[:, 512:1024], lhsT=vlhs, rhs=pt[:, 512:1024],
                            start=(kc == 0), stop=(kc == nkc - 1), skip_group_check=True,
                        )
                    # evacuate the accumulator to free the PSUM banks.
                    # high_priority pulls the whole normalize+project+store
                    # chain ahead of the next q-group's matmuls so the output
                    # DMA streams during attention instead of at the tail.
                    hp = tc.high_priority()
                    hp.__enter__()
                    ohu = op.tile([HD, 1024], F32, name="ohu", tag="ohu", bufs=2)
                    nc.vector.tensor_copy(out=ohu, in_=pv[0:HD, :])
                    r2 = sp.tile([1, 1024], F32R, name="r2", tag="rr", bufs=3)
                    with nc.allow_low_precision(reason="f32r reciprocal for PE broadcast"):
                        nc.vector.reciprocal(out=r2, in_=pv[HD : HD + 1, :])
                    ohn = op.tile([HD, 1024], F32R, name="ohn", tag="ohn", bufs=2)
                    for hf in range(2):
                        sl = slice(hf * 512, (hf + 1) * 512)
                        rbc = pss.tile([HD, 512], F32, space="PSUM", name="rbc", tag="s")
                        nc.tensor.matmul(rbc, lhsT=ones_t, rhs=r2[:, sl], start=True, stop=True)
                        nc.vector.tensor_mul(out=ohn[:, sl], in0=ohu[:, sl], in1=rbc)
                    hp.__exit__(None, None, None)
                    for hf in range(2):
                        sl = slice(hf * 512, (hf + 1) * 512)
                        osl = slice(q0 + hf * 512, q0 + (hf + 1) * 512)
                        for c in range(2):
                            pp = pss.tile([128, 512], F32, space="PSUM", name="pp", tag="s")
                            nc.tensor.matmul(
                                pp, lhsT=pwt_t[:, c * 128 : (c + 1) * 128], rhs=ohn[:, sl],
                                start=True, stop=True,
                            )
                            ot = op.tile([128, 512], F32, name="ot")
                            nc.vector.tensor_copy(out=ot, in_=pp)
                            nc.sync.dma_start(
                                out=out[c * 128 : (c + 1) * 128, osl], in_=ot
                            )

    _split_waits(nc)
    return nc


def _numpy_reference(x, block_types, gn_w, gn_b, qkv_w, qkv_b, proj_w, proj_b,
                     is_air, is_wood, is_leaves):
    """Pure-numpy fallback (degenerate case: no non-air keys)."""
    xf = x.reshape(B, C, N).astype(np.float64)
    xs = xf.reshape(B, GROUPS, GSIZE * N)
    mu = xs.mean(axis=2, keepdims=True)
    var = xs.var(axis=2, keepdims=True)
    h = ((xs - mu) / np.sqrt(var + EPS)).reshape(B, C, N)
    h = h * gn_w[None, :, None] + gn_b[None, :, None]
    qkv = np.einsum("oc,bcn->bon", qkv_w.astype(np.float64), h) + qkv_b[None, :, None]
    qkv = qkv.reshape(B, 3, HEADS, HD, N)
    q, k, v = qkv[:, 0], qkv[:, 1], qkv[:, 2]
    attn = np.einsum("bhdn,bhdm->bhnm", q, k) * (HD ** -0.5)
    bf = block_types.reshape(B, N)
    air = is_air[bf]; wood = is_wood[bf]; leaves = is_leaves[bf]
    attn = np.where(air[:, None, None, :] > 0, NEG, attn)
    wo = wood[:, :, None] * wood[:, None, :]
    lo = leaves[:, :, None] * leaves[:, None, :]
    mb = np.clip((wo + lo) * 2.0, 0.0, 10.0)
    pos = np.arange(N); ypos = (pos // W_) % H_
    vm = (np.abs(ypos[None, :] - ypos[:, None]) <= 2).astype(np.float64)
    vb = np.clip(wo * vm[None] * 1.5, 0.0, 10.0)
    attn = attn + (mb + vb)[:, None]
    attn = attn - attn.max(axis=-1, keepdims=True)
    e = np.exp(attn); p = e / e.sum(axis=-1, keepdims=True)
    o = np.einsum("bhnm,bhdm->bhdn", p, v).reshape(B, C, N)
    o = np.einsum("oc,bcn->bon", proj_w.astype(np.float64), o) + proj_b[None, :, None]
    return (xf + o).reshape(x.shape).astype(np.float32)


def kernel(x, block_types, gn_w, gn_b, qkv_w, qkv_b, proj_w, proj_b,
           is_air, is_wood, is_leaves):
    x = np.ascontiguousarray(np.asarray(x, dtype=np.float32))
    gn_w = np.asarray(gn_w, np.float32); gn_b = np.asarray(gn_b, np.float32)
    qkv_w = np.asarray(qkv_w, np.float32); qkv_b = np.asarray(qkv_b, np.float32)
    proj_w = np.asarray(proj_w, np.float32); proj_b = np.asarray(proj_b, np.float32)
    is_air = np.asarray(is_air, np.float32)
    is_wood = np.asarray(is_wood, np.float32)
    is_leaves = np.asarray(is_leaves, np.float32)
    bt = np.asarray(block_types).reshape(N).astype(np.int64)

    x2 = x.reshape(C, N)
    air = is_air[bt]; wood = is_wood[bt]; leaves = is_leaves[bt]
    idx = np.nonzero(air <= 0.0)[0]
    nk = len(idx)
    if nk == 0:
        return _numpy_reference(x, block_types, gn_w, gn_b, qkv_w, qkv_b,
                                proj_w, proj_b, is_air, is_wood, is_leaves)

    nk_pad = ((nk + 127) // 128) * 128
    nkc = nk_pad // 128
    idx_pad = np.concatenate([idx, np.full(nk_pad - nk, idx[0], np.int64)])

    # --- host-side O(N) feature prep ---
    ypos = ((np.arange(N) // W_) % H_).astype(np.int64)
    oneh = np.zeros((N, 16), np.float32); oneh[np.arange(N), ypos] = 1.0
    m16 = (np.abs(np.arange(16)[:, None] - np.arange(16)[None, :]) <= 2).astype(np.float32)
    lfeat = np.concatenate(
        [(2.0 * wood)[None], (2.0 * leaves)[None], 1.5 * wood[None] * oneh.T]
    ).astype(np.float32)                                   # [18, N]
    wood_k = wood[idx_pad]; leaves_k = leaves[idx_pad]
    mk = m16 @ oneh[idx_pad].T                             # [16, nk_pad]
    rfeat = np.concatenate(
        [wood_k[None], leaves_k[None], wood_k[None] * mk]
    ).astype(np.float32)                                   # [18, nk_pad]
    abias = np.zeros(nk_pad, np.float32); abias[nk:] = NEG
    abias = np.ascontiguousarray(abias.reshape(nkc, 128).T)  # [128, nkc]
    xc = np.ascontiguousarray(x2[:, idx_pad])

    gsel = np.zeros((128, 4), np.float32)
    gsel[np.arange(128), np.arange(128) // GSIZE] = 1.0
    ident33 = np.eye(33, dtype=np.float32)

    scale = HD ** -0.5
    shared = {
        "x2": x2, "xc": xc, "lfeat": lfeat, "rfeat": rfeat, "abias": abias,
        "gseli": gsel, "gselt": np.ascontiguousarray(gsel.T), "ident": ident33,
        "gnw": gn_w.reshape(C, 1), "gnb": gn_b.reshape(C, 1),
        "one32": np.ones((1, HD), np.float32),
        "onesk": np.ones((1, nk_pad), np.float32),
        "one32": np.ones((1, HD), np.float32),
        "onesk": np.ones((1, nk_pad), np.float32),
    }
    in_maps = []
    for hd_i in range(NCORES):
        r0 = hd_i * HD
        m = dict(shared)
        m["wq"] = np.ascontiguousarray((qkv_w[0 * C + r0 : 0 * C + r0 + HD] * scale).T)
        m["wk"] = np.ascontiguousarray(qkv_w[1 * C + r0 : 1 * C + r0 + HD].T)
        m["wv"] = np.ascontiguousarray(qkv_w[2 * C + r0 : 2 * C + r0 + HD].T)
        m["bq"] = np.ascontiguousarray((qkv_b[0 * C + r0 : 0 * C + r0 + HD] * scale)[:, None])
        m["bk"] = np.ascontiguousarray(qkv_b[1 * C + r0 : 1 * C + r0 + HD][:, None])
        m["bv"] = np.ascontiguousarray(qkv_b[2 * C + r0 : 2 * C + r0 + HD][:, None])
        m["pwt"] = np.ascontiguousarray(proj_w[:, r0 : r0 + HD].T)
        in_maps.append(m)

    if nk_pad not in _CACHE:
        _CACHE[nk_pad] = _build(nk_pad)
    nc = _CACHE[nk_pad]

    use_trace = TRACE
    if use_trace:
        import importlib.util
        if importlib.util.find_spec("antenv.axon_hooks") is None:
            use_trace = False
    res = run_bass_kernel_spmd(nc, in_maps, core_ids=list(range(NCORES)), trace=use_trace)
    LAST_RESULT["res"] = res

    acc = np.zeros((C, N), np.float32)
    for i in range(NCORES):
        acc += res.results[i]["o"]
    y = x2 + acc + proj_b[:, None]
    return y.reshape(B, C, D_, H_, W_).astype(np.float32)



# revision 26
# speedup vs baseline: 1.3534x; 1.3534x over previous
"""Trainium2 Bass kernel for MinecraftAwareAttention3D.

Full (unsharded) inputs in, full output out. One attention head per
NeuronCore (tensor parallel over num_heads=8); GroupNorm + QKV replicated
per core; per-head projection partials summed on the host.

Structure (v2):
  * Boost masks folded into QK^T as 18 extra contraction rows (bilinear).
  * Air keys compacted away on the host (softmax weight exactly 0).
  * No max-subtraction in softmax (logits are O(10)); padded keys killed
    with a -1e9 per-partition ACT bias on the last key chunk.
  * P stored as bf16. exp is split between ACT (exact, Exp) and DVE
    (Schraudolph: i16 = round(s*128/ln2 + b) bitcast to bf16, ~+-3%).
  * PV flipped: out[q, d] = P^T V via lhsT=P-chunk (bf16), rhs=V'^T chunk
    [128k, 33] -- 33-row matmuls accumulated over key chunks in one PSUM
    bank. A ones-row in V' makes column 32 the softmax denominator.
  * Normalization in [q, d] layout: per-partition reciprocal broadcast;
    PE transposes (bf16) restore [d, q] for the fused projection.
  * GroupNorm application runs on GPSIMD (Pool); stats on DVE under the
    x-load DMA shadow; rsqrt via quake seed + 2 Newton steps (no ACT
    table switch).
"""

import numpy as np

import concourse.bass as bass
import concourse.tile as tile
from concourse import mybir
from concourse.bass_utils import run_bass_kernel_spmd

F32 = mybir.dt.float32
F32R = mybir.dt.float32r
BF16 = mybir.dt.bfloat16
I16 = mybir.dt.int16
I32 = mybir.dt.int32
AF = mybir.ActivationFunctionType
ALU = mybir.AluOpType

B, C, D_, H_, W_ = 1, 256, 16, 16, 16
N = D_ * H_ * W_          # 4096 spatial positions
HEADS, HD = 8, 32
GROUPS = 8
GSIZE = C // GROUPS
EPS = 1e-5
NEG = -1e9
NF = HD + 18              # fused contraction depth
NCORES = 8

# Schraudolph exp in bf16 bits: i16 = round(x * 128/ln2 + SCH_B)
SCH_A = 184.66428327560596          # 2**7 / ln 2
SCH_B = 16250.232                   # 127*128 - 128*log2(1.0317) (centering)

TRACE = False
DEBUG = False
LAST_RESULT = {}
_CACHE = {}


def _split_waits(nc, max_waits=1):
    """This walrus build only encodes one sync wait per instruction; hoist
    extra waits onto same-engine NOPs inserted just before the instruction."""
    n = 0
    for f in nc.m.functions:
        for bb in f.blocks:
            new_insts = []
            for inst in bb.instructions:
                si = inst.sync_info
                if si is not None and si.on_wait and len(si.on_wait) > max_waits:
                    waits = list(si.on_wait)
                    si.on_wait = waits[-max_waits:]
                    for i in range(0, len(waits) - max_waits, max_waits):
                        n += 1
                        nop = mybir.InstNoOp(name=f"I-wsplit-{n}", ins=[], outs=[])
                        nop.engine = inst.engine
                        nop.sync_info = mybir.SyncInfo(
                            on_wait=waits[i : i + max_waits], on_update=[]
                        )
                        new_insts.append(nop)
                new_insts.append(inst)
            bb.instructions[:] = new_insts
    return n


# which key chunks each q-group exps on DVE (Schraudolph); the rest (incl.
# the padded last chunk) go through ACT's exact Exp. qg0 uses fewer: DVE is
# still busy with GN stats and k-path evacuation when attention starts.
DVE_KC = (
    (1, 3, 5, 7, 9, 11, 13, 15, 17, 19, 21, 22),
    (3, 5, 7, 9, 11, 13, 15, 19, 21),
    (3, 5, 7, 9, 11, 13, 15, 19, 21),
    (3, 5, 7, 9, 11, 13, 15, 19, 21),
)


def _build(nk_pad):
    nkc = nk_pad // 128
    ksl = [min(512, nk_pad - s) for s in range(0, nk_pad, 512)]
    dve_kc = [set(k for k in g if k < nkc - 1) for g in DVE_KC]

    nc = bass.Bass()

    # ---- I/O ----
    x2 = nc.dram_tensor("x2", [C, N], I16, kind="ExternalInput")
    xc = nc.dram_tensor("xc", [C, nk_pad], F32R, kind="ExternalInput")
    lfeat = nc.dram_tensor("lfeat", [18, N], F32R, kind="ExternalInput")
    rfeat = nc.dram_tensor("rfeat", [18, nk_pad], F32R, kind="ExternalInput")
    # cpk (plain f32 scalars): [gnw(0:2) gnb(2:4) abias(4:4+nkc) bq bk bv]
    CW = 7 + nkc
    cpk = nc.dram_tensor("cpk", [128, CW], F32, kind="ExternalInput")
    # cpkr (f32r, feeds the PE): [gsel(0:4) wq|wk|wv per half (4:196) gselt(196:324)]
    cpkr = nc.dram_tensor("cpkr", [128, 324], F32R, kind="ExternalInput")
    # idp (int16-coded bf16): ident128 [128,0:128], ident33 [33,128:161],
    # pwt [32,161:417]
    idp = nc.dram_tensor("idp", [128, 417], I16, kind="ExternalInput")
    onesk = nc.dram_tensor("onesk", [1, nk_pad], I16, kind="ExternalInput")
    out = nc.dram_tensor("o", [C, N], F32, kind="ExternalOutput")
    if DEBUG:
        d_qf = nc.dram_tensor("d_qf", [NF, N], F32, kind="ExternalOutput")
        d_kf = nc.dram_tensor("d_kf", [NF, nk_pad], F32, kind="ExternalOutput")
        d_vtb = nc.dram_tensor("d_vtb", [128, nkc * 33], I16, kind="ExternalOutput")
        d_ohn = nc.dram_tensor("d_ohn", [4, HD, 1024], I16, kind="ExternalOutput")
        d_vvb = nc.dram_tensor("d_vvb", [HD + 1, nk_pad], I16, kind="ExternalOutput")
        d_pv = nc.dram_tensor("d_pv", [4, 128, 8, 33], F32, kind="ExternalOutput")


    with tile.TileContext(nc) as tc:
        with (
            tc.tile_pool(name="consts", bufs=1) as cp,
            tc.tile_pool(name="live", bufs=1) as lp,
            tc.tile_pool(name="small", bufs=2) as sp,
            tc.tile_pool(name="ps_s", bufs=3, space="PSUM") as ps_s,
            tc.tile_pool(name="ps_pv", bufs=1, space="PSUM") as ps_pv,
        ):
            # ---- long-lived activations ----
            xt = [lp.tile([128, N], I16, name=f"xt{c}") for c in range(2)]
            xck = [lp.tile([128, nk_pad], F32R, name=f"xck{c}") for c in range(2)]
            qf = lp.tile([NF, N], F32R)            # Q' = [q*scale ; L]
            kf = lp.tile([NF, nk_pad], F32R)       # K' = [k ; R]
            vvb = lp.tile([HD + 1, nk_pad], BF16)  # [v ; ones]
            vtb = lp.tile([128, nkc, HD + 1], BF16)
            wsc = lp.tile([128, 2, 3, HD], F32R)   # a-scaled [wq|wk|wv] per half
            wqb = lp.tile([128, 2, HD], I16)       # bf16 a-scaled wq (x is bf16)

            # Warm the ACT exp table-set first.
            wz = cp.tile([1, 1], F32)
            nc.vector.memset(wz, 0.0)
            wy = cp.tile([1, 1], F32)
            nc.scalar.activation(out=wy, in_=wz, func=AF.Exp, bias=0.0, scale=1.0)

            # ================= Phase 1: loads + GroupNorm stats =================
            with tc.tile_pool(name="ps_g", bufs=1, space="PSUM") as ps_g:
                # cpk first (gates the GN scalar chain; own queue, lands early)
                cpk_t = cp.tile([128, CW], F32)
                nc.scalar.dma_start(out=cpk_t, in_=cpk[:, :])
                cpkr_t = cp.tile([128, 324], F32R)
                nc.scalar.dma_start(out=cpkr_t, in_=cpkr[:, :])
                # x halves next: everything else queues behind them on sync
                for s in range(4):
                    for c in range(2):
                        nc.sync.dma_start(
                            out=xt[c][:, s * 1024 : (s + 1) * 1024],
                            in_=x2[c * 128 : (c + 1) * 128, s * 1024 : (s + 1) * 1024],
                        )
                nc.sync.dma_start(out=qf[HD:NF, :], in_=lfeat[:, :])
                nc.sync.dma_start(out=kf[HD:NF, :], in_=rfeat[:, :])
                idp_t = cp.tile([128, 417], I16)
                nc.sync.dma_start(out=idp_t, in_=idp[:, :])
                # compacted keys stream last (k/v blocks consume in order)
                for s0 in range(0, nk_pad, 1024):
                    s1 = min(s0 + 1024, nk_pad)
                    for c in range(2):
                        nc.sync.dma_start(
                            out=xck[c][:, s0:s1],
                            in_=xc[c * 128 : (c + 1) * 128, s0:s1],
                        )
                nc.scalar.dma_start(out=vvb[HD : HD + 1, :].bitcast(I16), in_=onesk[:, :])

                # stats on DVE while x streams in
                stats2 = sp.tile([128, 4], F32R)
                st6 = [sp.tile([128, 8, 6], F32, name=f"st6_{c}") for c in range(2)]
                for s in range(8):
                    for c in range(2):
                        nc.vector.bn_stats(
                            out=st6[c][:, s, :], in_=xt[c][:, s * 512 : (s + 1) * 512].bitcast(BF16)
                        )
                for c in range(2):
                    mv = sp.tile([128, 2], F32, name=f"mv_{c}")
                    nc.vector.bn_aggr(out=mv, in_=st6[c])
                    m2 = sp.tile([128, 1], F32, name=f"m2_{c}")
                    nc.vector.tensor_mul(out=m2, in0=mv[:, 0:1], in1=mv[:, 0:1])
                    nc.vector.tensor_copy(
                        out=stats2[:, 2 * c : 2 * c + 1], in_=mv[:, 0:1]
                    )
                    nc.vector.tensor_add(
                        out=stats2[:, 2 * c + 1 : 2 * c + 2], in0=mv[:, 1:2], in1=m2
                    )

                hp = tc.high_priority()
                hp.__enter__()
                gstat = ps_g.tile([4, 2, 2], F32, space="PSUM", tag="g")
                nc.tensor.matmul(
                    gstat, lhsT=cpkr_t[:, 0:4], rhs=stats2,
                    start=True, stop=True,
                )
                # one [4,2] chain for both channel halves
                mu = sp.tile([4, 2], F32R, name="mu")
                nc.vector.tensor_scalar_mul(
                    out=mu, in0=gstat[:, :, 0], scalar1=1.0 / GSIZE
                )
                ve = sp.tile([4, 2], F32, name="ve")
                nc.vector.tensor_scalar(
                    out=ve, in0=gstat[:, :, 1], scalar1=1.0 / GSIZE, scalar2=EPS,
                    op0=ALU.mult, op1=ALU.add,
                )
                mum = sp.tile([4, 2], F32, name="mum")
                nc.vector.tensor_mul(out=mum, in0=mu, in1=mu)
                nc.vector.tensor_sub(out=ve, in0=ve, in1=mum)
                # rsqrt: quake seed + 2 Newton steps
                yi = sp.tile([4, 2], I32, name="yi")
                nc.vector.tensor_scalar(
                    out=yi, in0=ve.bitcast(I32), scalar1=1, scalar2=None,
                    op0=ALU.logical_shift_right,
                )
                nc.vector.tensor_scalar(
                    out=yi, in0=yi, scalar1=-1, scalar2=0x5F3759DF,
                    op0=ALU.mult, op1=ALU.add,
                )
                rs = sp.tile([4, 2], F32, name="rs")
                nc.vector.tensor_copy(out=rs, in_=yi.bitcast(F32))
                t2 = sp.tile([4, 2], F32, name="t2")
                for it in range(2):
                    nc.vector.tensor_mul(out=t2, in0=rs, in1=rs)
                    nc.vector.tensor_mul(out=t2, in0=t2, in1=ve)
                    nc.vector.tensor_scalar(
                        out=t2, in0=t2, scalar1=-0.5, scalar2=1.5,
                        op0=ALU.mult, op1=ALU.add,
                    )
                    nc.vector.tensor_mul(out=rs, in0=rs, in1=t2)
                musig = sp.tile([4, 2, 2], F32R, name="musig")
                nc.vector.tensor_copy(out=musig[:, :, 0], in_=mu)
                nc.vector.tensor_copy(out=musig[:, :, 1], in_=rs)
                bias_t = sp.tile([HD, 3], F32, name="bias_t")
                b_chs = []
                for c in range(2):
                    bc = ps_g.tile([128, 2], F32, space="PSUM", name=f"bc{c}", tag="g")
                    nc.tensor.matmul(
                        bc, lhsT=cpkr_t[0:4, 196:324],
                        rhs=musig[:, c, :], start=True, stop=True,
                    )
                    a_ch = sp.tile([128, 1], F32, name=f"a_ch_{c}")
                    nc.vector.tensor_mul(
                        out=a_ch, in0=cpk_t[:, 0 + c : 1 + c], in1=bc[:, 1:2]
                    )
                    b_ch = sp.tile([128, 1], F32R, name=f"b_ch_{c}")
                    nc.vector.tensor_mul(out=b_ch, in0=bc[:, 0:1], in1=a_ch)
                    nc.vector.tensor_sub(
                        out=b_ch, in0=cpk_t[:, 2 + c : 3 + c], in1=b_ch
                    )
                    b_chs.append(b_ch)
                    # fold a into the packed qkv weights: wsc = a * [wq wk wv]
                    nc.vector.tensor_scalar_mul(
                        out=wsc[:, c, :, :],
                        in0=cpkr_t[:, 4 + c * 96 : 4 + c * 96 + 96],
                        scalar1=a_ch,
                    )
                    nc.vector.tensor_scalar_mul(
                        out=wqb[:, c, :].bitcast(BF16),
                        in0=cpkr_t[:, 4 + c * 96 : 4 + c * 96 + HD],
                        scalar1=a_ch,
                    )
                # b correction: qkv bias += W^T b_ch (unscaled W)
                b2 = [sp.tile([128, 2], F32R, name=f"b2_{c}") for c in range(2)]
                for c in range(2):
                    nc.vector.tensor_copy(
                        out=b2[c], in_=b_chs[c].broadcast_to([128, 2])
                    )
                for pj in range(3):
                    bps = ps_g.tile([HD, 2], F32, space="PSUM", name=f"bp{pj}", tag="g")
                    for c in range(2):
                        nc.tensor.matmul(
                            bps,
                            lhsT=cpkr_t[:, 4 + c * 96 + pj * 32 : 4 + c * 96 + pj * 32 + HD],
                            rhs=b2[c],
                            start=(c == 0), stop=(c == 1),
                        )
                    nc.vector.tensor_add(
                        out=bias_t[:, pj : pj + 1],
                        in0=cpk_t[0:HD, 4 + nkc + pj : 5 + nkc + pj], in1=bps[:, 0:1],
                    )
                hp.__exit__(None, None, None)

            # ================= Phase 2: QKV + V' transpose =================
            if True:
                ident33 = idp_t[0:33, 128:161].bitcast(BF16)
                kvn = (nk_pad + 1023) // 1024
                for i in range(4):
                    paths = []
                    if i < kvn:
                        w_n = min(1024, nk_pad - i * 1024)
                        paths.append(("k", 1, kf, xck, w_n, i * 1024))
                    paths.append(("q", 0, qf, xt, 1024, i * 1024))
                    if i < kvn:
                        w_n = min(1024, nk_pad - i * 1024)
                        paths.append(("v", 2, vvb, xck, w_n, i * 1024))
                    for nm, pj, dst, src_t, w_n, s0 in paths:
                        ps = ps_s.tile([HD, 1024], F32, space="PSUM", name="qkv_ps", tag="st")
                        for cc in range(2):
                            for hf in range(0, w_n, 512):
                                hw = min(512, w_n - hf)
                                sl = slice(s0 + hf, s0 + hf + hw)
                                if nm == "q":
                                    lhsT = wqb[:, cc, :].bitcast(BF16)
                                    rhs = src_t[cc][:, sl].bitcast(BF16)
                                else:
                                    lhsT = wsc[:, cc, pj, :]
                                    rhs = src_t[cc][:, sl]
                                nc.tensor.matmul(
                                    ps[:, hf : hf + hw], lhsT=lhsT, rhs=rhs,
                                    start=(cc == 0), stop=(cc == 1),
                                )
                        hp = tc.high_priority()
                        hp.__enter__()
                        bias = bias_t[:, pj : pj + 1]
                        sl = slice(s0, s0 + w_n)
                        if nm == "k":
                            nc.vector.tensor_scalar_add(
                                out=dst[0:HD, sl], in0=ps[:, 0:w_n], scalar1=bias
                            )
                        else:  # q and v on ACT
                            nc.scalar.add(out=dst[0:HD, sl], in_=ps[:, 0:w_n], add=bias)
                        hp.__exit__(None, None, None)
                        if nm == "v":
                            # V'^T chunks for the flipped PV: transpose the
                            # key chunks this v-block just produced.
                            kc8 = 8 * i
                            nsub = min(8, nkc - kc8)
                            tps = ps_s.tile([128, 8, 34], BF16, space="PSUM", name="tr_ps", tag="st")
                            for j in range(nsub):
                                kc = kc8 + j
                                nc.tensor.transpose(
                                    tps[:, j, 0:33],
                                    in_=vvb[:, kc * 128 : (kc + 1) * 128],
                                    identity=ident33,
                                )
                            nc.vector.tensor_copy(
                                out=vtb[:, kc8 : kc8 + nsub, :],
                                in_=tps[:, 0:nsub, 0:33],
                            )

            if DEBUG:
                nc.scalar.dma_start(out=d_qf[:, :], in_=qf.bitcast(F32))
                nc.scalar.dma_start(out=d_kf[:, :], in_=kf.bitcast(F32))
                nc.scalar.dma_start(out=d_vtb[:, :], in_=vtb.bitcast(I16)[:, :, :])
                nc.scalar.dma_start(out=d_vvb[:, :], in_=vvb.bitcast(I16))
            # ========== Phase 3: attention (+ fused projection) ==========
            ident128 = idp_t[:, 0:128].bitcast(BF16)
            pwtb = idp_t[0:HD, 161:417].bitcast(BF16)
            with (
                tc.tile_pool(name="ps_pp", bufs=1, space="PSUM") as ps_pp,
                tc.tile_pool(name="ptp", bufs=3) as ptp,
                tc.tile_pool(name="opool", bufs=2) as op,
            ):
                def make_tail(qg, pv, last=False):
                    """Issue the normalize/transpose/project/store chain for a
                    finished q-group, interleaved into the next group's chunk
                    stream so PE never stalls on the DVE normalize."""
                    q0 = qg * 1024
                    hp = tc.high_priority()
                    hp.__enter__()
                    rden = sp.tile([128, 8], F32, name="rden", tag="rden", bufs=2)
                    if DEBUG:
                        dpv = op.tile([128, 8, 33], F32, name="dpv", tag="dpv", bufs=2)
                        nc.vector.tensor_copy(out=dpv, in_=pv[:, :, 0:33])
                        nc.scalar.dma_start(out=d_pv[qg], in_=dpv)
                    nc.vector.reciprocal(out=rden, in_=pv[:, :, 32])
                    sbn = op.tile([128, 8, HD], BF16, name="sbn", tag="sbn", bufs=2)
                    nc.vector.tensor_tensor(
                        out=sbn, in0=pv[:, :, 0:HD],
                        in1=rden.unsqueeze(2).broadcast_to([128, 8, HD]),
                        op=ALU.mult,
                    )
                    hp.__exit__(None, None, None)
                    pv2 = ps_s.tile([HD, 1024], BF16, space="PSUM", name="pv2", tag="st")
                    ohn = op.tile([HD, 1024], BF16, name="ohn", tag="ohn", bufs=2)
                    for hf in range(2):
                        sl = slice(hf * 512, (hf + 1) * 512)
                        for sub in range(hf * 4, hf * 4 + 4):
                            nc.tensor.transpose(
                                pv2[:, sub * 128 : (sub + 1) * 128],
                                in_=sbn[:, sub, :], identity=ident128,
                            )
                        nc.vector.tensor_copy(out=ohn[:, sl], in_=pv2[:, sl])
                        if DEBUG:
                            nc.scalar.dma_start(
                                out=d_ohn[qg, :, sl], in_=ohn[:, sl].bitcast(I16)
                            )
                        for c in range(2):
                            osl = slice(q0 + hf * 512, q0 + (hf + 1) * 512)
                            if last and (c + hf) % 2 == 1:
                                pp = ps_pv.tile([128, 512], F32, space="PSUM", name="pp2", tag="pv")
                            else:
                                pp = ps_pp.tile([128, 512], F32, space="PSUM", name="pp", tag="pp")
                            nc.tensor.matmul(
                                pp, lhsT=pwtb[:, c * 128 : (c + 1) * 128],
                                rhs=ohn[:, sl], start=True, stop=True,
                            )
                            ot = op.tile([128, 512], F32, name="ot", tag="ot", bufs=4)
                            if (qg + c) % 2 == 0:
                                nc.scalar.add(out=ot, in_=pp, add=0.0)
                            else:
                                nc.vector.tensor_copy(out=ot, in_=pp)
                            eng = nc.sync if (qg + 2 * hf + c) % 2 == 0 else nc.scalar
                            eng.dma_start(
                                out=out[c * 128 : (c + 1) * 128, osl], in_=ot
                            )

                def issue_S(qg, kc):
                    st = ps_s.tile([128, 1024], F32, space="PSUM", name="st", tag="st")
                    q0 = qg * 1024
                    lhs = kf[:, kc * 128 : (kc + 1) * 128]
                    nc.tensor.matmul(
                        st[:, 0:512], lhsT=lhs, rhs=qf[:, q0 : q0 + 512],
                        start=True, stop=True,
                    )
                    nc.tensor.matmul(
                        st[:, 512:1024], lhsT=lhs, rhs=qf[:, q0 + 512 : q0 + 1024],
                        start=True, stop=True,
                    )
                    return st

                def issue_exp(qg, kc, st):
                    pt = ptp.tile([128, 1024], I16, name="pt", tag="pt")
                    if kc in dve_kc[qg]:
                        nc.vector.tensor_scalar(
                            out=pt, in0=st, scalar1=SCH_A, scalar2=SCH_B,
                            op0=ALU.mult, op1=ALU.add,
                        )
                    else:
                        nc.scalar.activation(
                            out=pt.bitcast(BF16), in_=st, func=AF.Exp,
                            bias=cpk_t[:, 4 + kc : 5 + kc], scale=1.0,
                        )
                    return pt

                def issue_pv(pv, kc, pt, first, last):
                    # partial-bank start=True matmuls lose data on HW (probe5);
                    # the bank is memset-zeroed instead and all PVs accumulate.
                    ptb = pt.bitcast(BF16)
                    for sub in range(8):
                        nc.tensor.matmul(
                            pv[:, sub, 0:33],
                            lhsT=ptb[:, sub * 128 : (sub + 1) * 128],
                            rhs=vtb[:, kc, :],
                            start=False, stop=last,
                            skip_group_check=True,
                        )

                korder = list(range(nkc))
                if nkc > 13:  # hoist the padded (ACT-bias) chunk off the group end
                    korder = korder[:12] + [nkc - 1] + korder[12:-1]

                # one flat software pipeline across all q-groups:
                # iteration j issues S(j+2) | exp(j+1) | [tail of the group
                # that PV(j-1) just finished] | PV(j).
                NQG = 4
                NJ = NQG * nkc
                def jqk(j):
                    return j // nkc, korder[j % nkc]

                pvs = {}
                sts = {}
                pts = {}

                def flat_S(j):
                    qg, kc = jqk(j)
                    sts[j] = issue_S(qg, kc)

                def flat_exp(j):
                    qg, kc = jqk(j)
                    pts[j] = issue_exp(qg, kc, sts.pop(j))

                flat_S(0); flat_S(1); flat_exp(0)
                for j in range(NJ):
                    if j + 2 < NJ:
                        flat_S(j + 2)
                    if j + 1 < NJ:
                        flat_exp(j + 1)
                    qg, kc = jqk(j)
                    i = j % nkc
                    if i == 0:
                        if qg > 0:
                            make_tail(qg - 1, pvs.pop(qg - 1), last=False)
                        pvs[qg] = ps_pv.tile([128, 8, 64], F32, space="PSUM", name="pv", tag="pv")
                        nc.vector.memset(pvs[qg], 0.0)
                    issue_pv(pvs[qg], kc, pts.pop(j), first=(i == 0), last=(i == nkc - 1))
                make_tail(NQG - 1, pvs.pop(NQG - 1), last=True)

    _split_waits(nc)
    return nc


def _numpy_reference(x, block_types, gn_w, gn_b, qkv_w, qkv_b, proj_w, proj_b,
                     is_air, is_wood, is_leaves):
    """Pure-numpy fallback (degenerate case: no non-air keys)."""
    xf = x.reshape(B, C, N).astype(np.float64)
    xs = xf.reshape(B, GROUPS, GSIZE * N)
    mu = xs.mean(axis=2, keepdims=True)
    var = xs.var(axis=2, keepdims=True)
    h = ((xs - mu) / np.sqrt(var + EPS)).reshape(B, C, N)
    h = h * gn_w[None, :, None] + gn_b[None, :, None]
    qkv = np.einsum("oc,bcn->bon", qkv_w.astype(np.float64), h) + qkv_b[None, :, None]
    qkv = qkv.reshape(B, 3, HEADS, HD, N)
    q, k, v = qkv[:, 0], qkv[:, 1], qkv[:, 2]
    attn = np.einsum("bhdn,bhdm->bhnm", q, k) * (HD ** -0.5)
    bf = block_types.reshape(B, N)
    air = is_air[bf]; wood = is_wood[bf]; leaves = is_leaves[bf]
    attn = np.where(air[:, None, None, :] > 0, NEG, attn)
    wo = wood[:, :, None] * wood[:, None, :]
    lo = leaves[:, :, None] * leaves[:, None, :]
    mb = np.clip((wo + lo) * 2.0, 0.0, 10.0)
    pos = np.arange(N); ypos = (pos // W_) % H_
    vm = (np.abs(ypos[None, :] - ypos[:, None]) <= 2).astype(np.float64)
    vb = np.clip(wo * vm[None] * 1.5, 0.0, 10.0)
    attn = attn + (mb + vb)[:, None]
    attn = attn - attn.max(axis=-1, keepdims=True)
    e = np.exp(attn); p = e / e.sum(axis=-1, keepdims=True)
    o = np.einsum("bhnm,bhdm->bhdn", p, v).reshape(B, C, N)
    o = np.einsum("oc,bcn->bon", proj_w.astype(np.float64), o) + proj_b[None, :, None]
    return (xf + o).reshape(x.shape).astype(np.float32)


def _bf16_bits(a):
    """float32 array -> round-to-nearest-even bf16 bit pattern as int16."""
    b = np.ascontiguousarray(a, dtype=np.float32).view(np.uint32)
    b = (b + 0x7FFF + ((b >> 16) & 1)) >> 16
    return b.astype(np.uint16).view(np.int16)


def kernel(x, block_types, gn_w, gn_b, qkv_w, qkv_b, proj_w, proj_b,
           is_air, is_wood, is_leaves):
    x = np.ascontiguousarray(np.asarray(x, dtype=np.float32))
    gn_w = np.asarray(gn_w, np.float32); gn_b = np.asarray(gn_b, np.float32)
    qkv_w = np.asarray(qkv_w, np.float32); qkv_b = np.asarray(qkv_b, np.float32)
    proj_w = np.asarray(proj_w, np.float32); proj_b = np.asarray(proj_b, np.float32)
    is_air = np.asarray(is_air, np.float32)
    is_wood = np.asarray(is_wood, np.float32)
    is_leaves = np.asarray(is_leaves, np.float32)
    bt = np.asarray(block_types).reshape(N).astype(np.int64)

    x2 = x.reshape(C, N)
    air = is_air[bt]; wood = is_wood[bt]; leaves = is_leaves[bt]
    idx = np.nonzero(air <= 0.0)[0]
    nk = len(idx)
    if nk == 0:
        return _numpy_reference(x, block_types, gn_w, gn_b, qkv_w, qkv_b,
                                proj_w, proj_b, is_air, is_wood, is_leaves)

    nk_pad = ((nk + 127) // 128) * 128
    nkc = nk_pad // 128
    idx_pad = np.concatenate([idx, np.full(nk_pad - nk, idx[0], np.int64)])

    # --- host-side O(N) feature prep ---
    ypos = ((np.arange(N) // W_) % H_).astype(np.int64)
    oneh = np.zeros((N, 16), np.float32); oneh[np.arange(N), ypos] = 1.0
    m16 = (np.abs(np.arange(16)[:, None] - np.arange(16)[None, :]) <= 2).astype(np.float32)
    lfeat = np.concatenate(
        [(2.0 * wood)[None], (2.0 * leaves)[None], 1.5 * wood[None] * oneh.T]
    ).astype(np.float32)                                   # [18, N]
    wood_k = wood[idx_pad]; leaves_k = leaves[idx_pad]
    mk = m16 @ oneh[idx_pad].T                             # [16, nk_pad]
    rfeat = np.concatenate(
        [wood_k[None], leaves_k[None], wood_k[None] * mk]
    ).astype(np.float32)                                   # [18, nk_pad]
    abias = np.zeros(nk_pad, np.float32); abias[nk:] = NEG
    abias = np.ascontiguousarray(abias.reshape(nkc, 128).T)  # [128, nkc]
    xcv = np.ascontiguousarray(x2[:, idx_pad])
    x2b = np.ascontiguousarray(_bf16_bits(x2))

    gsel = np.zeros((128, 4), np.float32)
    gsel[np.arange(128), np.arange(128) // GSIZE] = 1.0

    scale = HD ** -0.5
    CW = 7 + nkc
    cpk = np.zeros((128, CW), np.float32)
    cpk[:, 0] = gn_w[0:128]; cpk[:, 1] = gn_w[128:256]
    cpk[:, 2] = gn_b[0:128]; cpk[:, 3] = gn_b[128:256]
    cpk[:, 4 : 4 + nkc] = abias

    in_maps = []
    for hd_i in range(NCORES):
        r0 = hd_i * HD
        wq = (qkv_w[0 * C + r0 : 0 * C + r0 + HD] * scale).T   # [C, HD]
        wk = qkv_w[1 * C + r0 : 1 * C + r0 + HD].T
        wv = qkv_w[2 * C + r0 : 2 * C + r0 + HD].T
        cpk_i = cpk.copy()
        cpk_i[0:HD, 4 + nkc] = qkv_b[0 * C + r0 : 0 * C + r0 + HD] * scale
        cpk_i[0:HD, 5 + nkc] = qkv_b[1 * C + r0 : 1 * C + r0 + HD]
        cpk_i[0:HD, 6 + nkc] = qkv_b[2 * C + r0 : 2 * C + r0 + HD]
        cpkr_i = np.zeros((128, 324), np.float32)
        cpkr_i[:, 0:4] = gsel
        for cc in range(2):
            rows = slice(cc * 128, (cc + 1) * 128)
            cpkr_i[:, 4 + cc * 96 : 4 + cc * 96 + HD] = wq[rows]
            cpkr_i[:, 4 + cc * 96 + 32 : 4 + cc * 96 + 32 + HD] = wk[rows]
            cpkr_i[:, 4 + cc * 96 + 64 : 4 + cc * 96 + 64 + HD] = wv[rows]
        cpkr_i[0:4, 196:324] = gsel.T[:, 0:128]
        idp_i = np.zeros((128, 417), np.int16)
        idp_i[:, 0:128] = _bf16_bits(np.eye(128, dtype=np.float32))
        idp_i[0:33, 128:161] = _bf16_bits(np.eye(33, dtype=np.float32))
        idp_i[0:HD, 161:417] = _bf16_bits(np.ascontiguousarray(proj_w[:, r0 : r0 + HD].T))
        m = {
            "x2": x2b, "xc": xcv, "lfeat": lfeat, "rfeat": rfeat,
            "cpk": cpk_i, "cpkr": cpkr_i, "idp": idp_i,
            "onesk": np.full((1, nk_pad), 0x3F80, np.int16),
        }
        in_maps.append(m)

    if nk_pad not in _CACHE:
        _CACHE[nk_pad] = _build(nk_pad)
    nc = _CACHE[nk_pad]

    use_trace = TRACE
    if use_trace:
        import importlib.util
        if importlib.util.find_spec("antenv.axon_hooks") is None:
            use_trace = False
    res = run_bass_kernel_spmd(nc, in_maps, core_ids=list(range(NCORES)), trace=use_trace)
    LAST_RESULT["res"] = res

    acc = np.zeros((C, N), np.float32)
    for i in range(NCORES):
        acc += res.results[i]["o"]
    y = x2 + acc + proj_b[:, None]
    return y.reshape(B, C, D_, H_, W_).astype(np.float32)
